# revision 1
# baseline (speedup 1.0000x reference)
"""MLA (multi-head latent attention) prefill kernel for 8 TRN2 NeuronCores.

Sharding: 4 head-groups x 2 batches. Core c: batch = c // 4, head-group g = c % 4
(4 heads each). Each core computes its batch's down-projections + RMSNorm,
its 4 heads' q_up / attention / ctx, and a partial output projection
(out_w column slice). Host sums the 4 partials per batch (TP unshard).

Device compute uses fp32 storage; matmuls run as float32r (full-rate PE).
All weight transposes are done on host (numpy) so the device only transposes
activations (PE transpose via identity).
"""

import sys
import os

for _p in ("/opt/trn_rl_repo", "/root/.axon_site/_ro/trn_rl_repo"):
    if os.path.isdir(_p) and _p not in sys.path:
        sys.path.insert(0, _p)

import numpy as np

import concourse.bass as bass
import concourse.bacc as bacc
import concourse.tile as tile
import concourse.mybir as mybir
from concourse.bass_utils import run_bass_kernel_spmd
from concourse.masks import make_identity

F32 = mybir.dt.float32
F32R = mybir.dt.float32r

DIM, H, Q_RANK, KV_RANK = 2048, 16, 768, 512
QK_STATIC, QK_ROT, V_DIM = 128, 64, 128
QK_TOTAL = QK_STATIC + QK_ROT
BS, SEQ = 2, 1024
HPC = 4          # heads per core
N_CORES = 8
P = 128
NSB = SEQ // P   # 8 s-blocks
NKD = DIM // P   # 16 d-chunks

MM_F32R = True   # bitcast matmul operands to float32r (1 cyc/row vs 4)


MMDT = F32R if MM_F32R else F32


def _mm(ap):
    return ap


def build_kernel():
    nc = bacc.Bacc("TRN2", target_bir_lowering=False, debug=False)

    def din(name, shape, dt=F32):
        return nc.dram_tensor(name, list(shape), dt, kind="ExternalInput")

    xT = din("xT", (DIM, SEQ), MMDT)
    mask = din("mask", (SEQ, SEQ))
    wqdT = din("wqdT", (DIM, Q_RANK), MMDT)
    qdb = din("qdb", (1, Q_RANK), MMDT)
    wkvdT = din("wkvdT", (DIM, KV_RANK + QK_ROT), MMDT)
    kvdb = din("kvdb", (1, KV_RANK + QK_ROT), MMDT)
    wqupT = din("wqupT", (Q_RANK, HPC * QK_TOTAL), MMDT)
    qub = din("qub", (1, HPC * QK_TOTAL), MMDT)
    wkT = din("wkT", (HPC, KV_RANK, QK_STATIC), MMDT)
    wvT = din("wvT", (HPC, KV_RANK, V_DIM), MMDT)
    woutT = din("woutT", (HPC * V_DIM, DIM), MMDT)
    outb = din("outb", (1, DIM), MMDT)
    cosf = din("cosf", (SEQ, QK_ROT))
    sinhr = din("sinhr", (SEQ, QK_ROT))
    ones_in = din("ones_in", (1, P), MMDT)

    out_p = nc.dram_tensor("out_p", [SEQ, DIM], F32, kind="ExternalOutput")

    RV = KV_RANK + QK_ROT  # 576

    with tile.TileContext(nc) as tc:
        import contextlib
        ctx = contextlib.ExitStack()
        with ctx:
            const = ctx.enter_context(tc.tile_pool(name="const", bufs=1))
            persist = ctx.enter_context(tc.tile_pool(name="persist", bufs=1))
            scv = ctx.enter_context(tc.tile_pool(name="scratch_vec", bufs=4))
            ppt = ctx.enter_context(tc.tile_pool(name="psum_t", bufs=2, space="PSUM"))

            ident = const.tile([P, P], F32, tag="ident")
            make_identity(nc, ident[:])
            ones_row = const.tile([1, P], MMDT, tag="ones")
            nc.sync.dma_start(ones_row[:], ones_in[:])

            def load_const(name, src, shape, dt=F32):
                t = const.tile(list(shape), dt, name=name, tag=name)
                nc.sync.dma_start(t[:], src[:])
                return t

            t_qdb = load_const("qdb", qdb, (1, Q_RANK), MMDT)
            t_kvdb = load_const("kvdb", kvdb, (1, RV), MMDT)
            t_qub = load_const("qub", qub, (1, HPC * QK_TOTAL), MMDT)
            t_outb = load_const("outb", outb, (1, DIM), MMDT)

            t_cos, t_sin = [], []
            for sb in range(NSB):
                c = const.tile([P, QK_ROT], F32, name=f"cos{sb}", tag=f"cos{sb}")
                s = const.tile([P, QK_ROT], F32, name=f"sin{sb}", tag=f"sin{sb}")
                nc.sync.dma_start(c[:], cosf[sb * P:(sb + 1) * P, :])
                nc.sync.dma_start(s[:], sinhr[sb * P:(sb + 1) * P, :])
                t_cos.append(c)
                t_sin.append(s)

            # persistent activation tensors (per-partition: 4+16+4+16+16 = 56KB)
            kvnT = [persist.tile([P, SEQ], MMDT, name=f"kvnT{ct}", tag=f"kvnT{ct}")
                    for ct in range(4)]
            krT = persist.tile([QK_ROT, SEQ], MMDT, name="krT", tag="krT")
            qsT = [persist.tile([P, SEQ], MMDT, name=f"qsT{h}", tag=f"qsT{h}")
                   for h in range(HPC)]
            qrT = [persist.tile([QK_ROT, SEQ], MMDT, name=f"qrT{h}", tag=f"qrT{h}")
                   for h in range(HPC)]
            ctxT = [persist.tile([P, SEQ], MMDT, name=f"ctxT{h}", tag=f"ctxT{h}")
                    for h in range(HPC)]

            def rmsnorm_stats(pool, ps_list, widths, inv_n):
                ssqs = []
                for psrc, w in zip(ps_list, widths):
                    sq = pool.tile([P, w], F32, name="sq", tag="sq")
                    ssq = scv.tile([P, 1], F32, name="ssq", tag="ssq")
                    nc.scalar.activation(sq[:], psrc, mybir.ActivationFunctionType.Square,
                                         accum_out=ssq[:])
                    ssqs.append(ssq)
                tot = ssqs[0]
                if len(ssqs) > 1:
                    tot = scv.tile([P, 1], F32, name="ssq_tot", tag="ssq_tot")
                    nc.vector.tensor_tensor(tot[:], ssqs[0][:], ssqs[1][:],
                                            op=mybir.AluOpType.add)
                mseps = scv.tile([P, 1], F32, name="mseps", tag="mseps")
                nc.vector.tensor_scalar(mseps[:], tot[:], inv_n, 1e-6,
                                        op0=mybir.AluOpType.mult,
                                        op1=mybir.AluOpType.add)
                rinv = scv.tile([P, 1], F32, name="rinv", tag="rinv")
                nc.vector.reciprocal(rinv[:], mseps[:])
                rstd = scv.tile([P, 1], F32, name="rstd", tag="rstd")
                nc.scalar.sqrt(rstd[:], rinv[:])
                return rstd

            def rope(pool, dst, src_ap, sb):
                lo, hi = (0, 32), (32, 64)
                for (a0, a1), (b0, b1) in ((lo, hi), (hi, lo)):
                    m1 = pool.tile([P, 32], F32, name="rope_m1", tag="rope_m1")
                    m2 = pool.tile([P, 32], F32, name="rope_m2", tag="rope_m2")
                    nc.vector.tensor_tensor(m1[:], src_ap[:, a0:a1], t_cos[sb][:, a0:a1],
                                            op=mybir.AluOpType.mult)
                    nc.vector.tensor_tensor(m2[:], src_ap[:, b0:b1], t_sin[sb][:, a0:a1],
                                            op=mybir.AluOpType.mult)
                    nc.vector.tensor_tensor(dst[:, a0:a1], m1[:], m2[:],
                                            op=mybir.AluOpType.add)

            def transpose_to(dst_ap, src_ap, rows, cols):
                pst = ppt.tile([cols, rows], F32, name="trans", tag="trans")
                nc.tensor.transpose(pst[:], src_ap, ident[:rows, :rows])
                nc.vector.tensor_copy(dst_ap, pst[:])

            def load_x_slice(pool, sb):
                xs = pool.tile([P, NKD * P], MMDT, name="xsl", tag="xsl")
                for k in range(NKD):
                    nc.sync.dma_start(xs[:, k * P:(k + 1) * P],
                                      xT[k * P:(k + 1) * P, sb * P:(sb + 1) * P])
                return xs

            # ---------- PHASE 1: kv path ----------
            with tc.tile_pool(name="wkv_pool", bufs=1) as wp1, \
                 tc.tile_pool(name="sc1", bufs=2) as sc1, \
                 tc.tile_pool(name="pp1", bufs=4, space="PSUM") as pp:
                wkv = []
                for k in range(NKD):
                    t = wp1.tile([P, RV], MMDT, name=f"wkvd{k}", tag=f"wkvd{k}")
                    nc.sync.dma_start(t[:], wkvdT[k * P:(k + 1) * P, :])
                    wkv.append(t)

                for sb in range(NSB):
                    xs = load_x_slice(sc1, sb)
                    psA = pp.tile([P, KV_RANK], F32, name="ps_kvA", tag="ps")
                    psB = pp.tile([P, QK_ROT], F32, name="ps_kvB", tag="ps")
                    for k in range(NKD):
                        nc.tensor.matmul(psA[:], _mm(xs[:, k * P:(k + 1) * P]),
                                         _mm(wkv[k][:, :KV_RANK]),
                                         start=(k == 0), stop=False)
                        nc.tensor.matmul(psB[:], _mm(xs[:, k * P:(k + 1) * P]),
                                         _mm(wkv[k][:, KV_RANK:]),
                                         start=(k == 0), stop=False)
                    nc.tensor.matmul(psA[:], _mm(ones_row[:]), _mm(t_kvdb[:, :KV_RANK]),
                                     start=False, stop=True)
                    nc.tensor.matmul(psB[:], _mm(ones_row[:]), _mm(t_kvdb[:, KV_RANK:]),
                                     start=False, stop=True)

                    rstd = rmsnorm_stats(sc1, [psA[:]], [KV_RANK], 1.0 / KV_RANK)
                    kvn = sc1.tile([P, KV_RANK], F32, name="kvn", tag="kvn")
                    nc.vector.tensor_scalar(kvn[:], psA[:], rstd[:], None,
                                            op0=mybir.AluOpType.mult)
                    for ct in range(4):
                        transpose_to(kvnT[ct][:, sb * P:(sb + 1) * P],
                                     kvn[:, ct * P:(ct + 1) * P], P, P)
                    kr = sc1.tile([P, QK_ROT], F32, name="kr", tag="kr")
                    rope(sc1, kr, psB, sb)
                    transpose_to(krT[:, sb * P:(sb + 1) * P], kr[:], P, QK_ROT)

            # ---------- PHASE 2: q path ----------
            with tc.tile_pool(name="wq_pool", bufs=1) as wp2, \
                 tc.tile_pool(name="sc2", bufs=2) as sc2, \
                 tc.tile_pool(name="pp2", bufs=4, space="PSUM") as pp:
                wqd = []
                for k in range(NKD):
                    t = wp2.tile([P, Q_RANK], MMDT, name=f"wqd{k}", tag=f"wqd{k}")
                    nc.sync.dma_start(t[:], wqdT[k * P:(k + 1) * P, :])
                    wqd.append(t)
                wqu = []
                for k in range(Q_RANK // P):
                    t = wp2.tile([P, HPC * QK_TOTAL], MMDT, name=f"wqu{k}", tag=f"wqu{k}")
                    nc.sync.dma_start(t[:], wqupT[k * P:(k + 1) * P, :])
                    wqu.append(t)

                for sb in range(NSB):
                    xs = load_x_slice(sc2, sb)
                    psA = pp.tile([P, 512], F32, name="ps_qA", tag="ps")
                    psB = pp.tile([P, Q_RANK - 512], F32, name="ps_qB", tag="ps")
                    for k in range(NKD):
                        nc.tensor.matmul(psA[:], _mm(xs[:, k * P:(k + 1) * P]),
                                         _mm(wqd[k][:, :512]), start=(k == 0), stop=False)
                        nc.tensor.matmul(psB[:], _mm(xs[:, k * P:(k + 1) * P]),
                                         _mm(wqd[k][:, 512:]), start=(k == 0), stop=False)
                    nc.tensor.matmul(psA[:], _mm(ones_row[:]), _mm(t_qdb[:, :512]),
                                     start=False, stop=True)
                    nc.tensor.matmul(psB[:], _mm(ones_row[:]), _mm(t_qdb[:, 512:]),
                                     start=False, stop=True)

                    rstd = rmsnorm_stats(sc2, [psA[:], psB[:]], [512, Q_RANK - 512],
                                         1.0 / Q_RANK)
                    qn = sc2.tile([P, Q_RANK], F32, name="qn", tag="qn")
                    nc.vector.tensor_scalar(qn[:, :512], psA[:], rstd[:], None,
                                            op0=mybir.AluOpType.mult)
                    nc.vector.tensor_scalar(qn[:, 512:], psB[:], rstd[:], None,
                                            op0=mybir.AluOpType.mult)

                    qnT = []
                    for k in range(Q_RANK // P):
                        t = sc2.tile([P, P], MMDT, name=f"qnT{k}", tag=f"qnT{k}")
                        transpose_to(t[:], qn[:, k * P:(k + 1) * P], P, P)
                        qnT.append(t)

                    NQ = HPC * QK_TOTAL  # 768
                    psC = pp.tile([P, 512], F32, name="ps_quA", tag="ps")
                    psD = pp.tile([P, NQ - 512], F32, name="ps_quB", tag="ps")
                    for k in range(Q_RANK // P):
                        nc.tensor.matmul(psC[:], _mm(qnT[k][:]), _mm(wqu[k][:, :512]),
                                         start=(k == 0), stop=False)
                        nc.tensor.matmul(psD[:], _mm(qnT[k][:]), _mm(wqu[k][:, 512:]),
                                         start=(k == 0), stop=False)
                    nc.tensor.matmul(psC[:], _mm(ones_row[:]), _mm(t_qub[:, :512]),
                                     start=False, stop=True)
                    nc.tensor.matmul(psD[:], _mm(ones_row[:]), _mm(t_qub[:, 512:]),
                                     start=False, stop=True)

                    q_sb = sc2.tile([P, NQ], F32, name="q_sb", tag="q_sb")
                    nc.vector.tensor_copy(q_sb[:, :512], psC[:])
                    nc.vector.tensor_copy(q_sb[:, 512:], psD[:])

                    for h in range(HPC):
                        base = h * QK_TOTAL
                        transpose_to(qsT[h][:, sb * P:(sb + 1) * P],
                                     q_sb[:, base:base + QK_STATIC], P, P)
                        qr = sc2.tile([P, QK_ROT], F32, name="qr", tag="qr")
                        rope(sc2, qr, q_sb[:, base + QK_STATIC:base + QK_TOTAL], sb)
                        transpose_to(qrT[h][:, sb * P:(sb + 1) * P], qr[:], P, QK_ROT)

            # ---------- PHASE 3: attention ----------
            with tc.tile_pool(name="attn_pool", bufs=1) as ap, \
                 tc.tile_pool(name="attn_sc", bufs=2) as asc, \
                 tc.tile_pool(name="probp", bufs=2) as probp, \
                 tc.tile_pool(name="pp3", bufs=4, space="PSUM") as pp, \
                 tc.tile_pool(name="pp3c", bufs=1, space="PSUM") as ppc:
                wk_t, wv_t = [], []
                for h in range(HPC):
                    for cc in range(4):
                        tk = ap.tile([P, QK_STATIC], MMDT, name=f"wk{h}_{cc}",
                                     tag=f"wk{h}_{cc}")
                        nc.sync.dma_start(tk[:], wkT[h, cc * P:(cc + 1) * P, :])
                        tv = ap.tile([P, V_DIM], MMDT, name=f"wv{h}_{cc}",
                                     tag=f"wv{h}_{cc}")
                        nc.sync.dma_start(tv[:], wvT[h, cc * P:(cc + 1) * P, :])
                        wk_t.append(tk)
                        wv_t.append(tv)

                for h in range(HPC):
                    keff = asc.tile([P, SEQ], MMDT, name="keff", tag="keff")
                    veff = asc.tile([P, SEQ], F32, name="veff", tag="veff")
                    for tb in range(2):
                        psk = pp.tile([P, 512], F32, name="ps_keff", tag="ps")
                        psv = pp.tile([P, 512], F32, name="ps_veff", tag="ps")
                        for cc in range(4):
                            nc.tensor.matmul(psk[:], _mm(wk_t[h * 4 + cc][:]),
                                             _mm(kvnT[cc][:, tb * 512:(tb + 1) * 512]),
                                             start=(cc == 0), stop=(cc == 3))
                            nc.tensor.matmul(psv[:], _mm(wv_t[h * 4 + cc][:]),
                                             _mm(kvnT[cc][:, tb * 512:(tb + 1) * 512]),
                                             start=(cc == 0), stop=(cc == 3))
                        nc.vector.tensor_copy(keff[:, tb * 512:(tb + 1) * 512], psk[:])
                        nc.vector.tensor_copy(veff[:, tb * 512:(tb + 1) * 512], psv[:])

                    veffT = asc.tile([P, SEQ], MMDT, name="veffT", tag="veffT")
                    for tcn in range(NSB):
                        transpose_to(veffT[:, tcn * P:(tcn + 1) * P],
                                     veff[:, tcn * P:(tcn + 1) * P], P, P)

                    for shalf in range(2):
                        pT = [probp.tile([P, 512], MMDT, name=f"pT{tcn}", tag=f"pT{tcn}")
                              for tcn in range(NSB)]
                        for sb4 in range(4):
                            sb = shalf * 4 + sb4
                            ps0 = pp.tile([P, 512], F32, name="ps_sc0", tag="ps")
                            ps1 = pp.tile([P, 512], F32, name="ps_sc1", tag="ps")
                            for tb, pstb in enumerate((ps0, ps1)):
                                nc.tensor.matmul(pstb[:],
                                                 _mm(qsT[h][:, sb * P:(sb + 1) * P]),
                                                 _mm(keff[:, tb * 512:(tb + 1) * 512]),
                                                 start=True, stop=False)
                                nc.tensor.matmul(pstb[:],
                                                 _mm(qrT[h][:, sb * P:(sb + 1) * P]),
                                                 _mm(krT[:, tb * 512:(tb + 1) * 512]),
                                                 start=False, stop=True)
                            mt = asc.tile([P, SEQ], F32, name="mask_t", tag="mask_t")
                            nc.sync.dma_start(mt[:], mask[sb * P:(sb + 1) * P, :])
                            scs = asc.tile([P, SEQ], F32, name="scores", tag="scores")
                            nc.vector.tensor_tensor(scs[:, :512], ps0[:], mt[:, :512],
                                                    op=mybir.AluOpType.add)
                            nc.vector.tensor_tensor(scs[:, 512:], ps1[:], mt[:, 512:],
                                                    op=mybir.AluOpType.add)
                            mx = scv.tile([P, 1], F32, name="mx", tag="mx")
                            nc.vector.reduce_max(mx[:], scs[:],
                                                 axis=mybir.AxisListType.X)
                            negmax = scv.tile([P, 1], F32, name="negmax", tag="negmax")
                            nc.vector.tensor_scalar(negmax[:], mx[:], -1.0, None,
                                                    op0=mybir.AluOpType.mult)
                            probs = asc.tile([P, SEQ], F32, name="probs", tag="probs")
                            rowsum = scv.tile([P, 1], F32, name="rowsum", tag="rowsum")
                            nc.scalar.activation(probs[:], scs[:],
                                                 mybir.ActivationFunctionType.Exp,
                                                 bias=negmax[:], accum_out=rowsum[:])
                            logsum = scv.tile([P, 1], F32, name="logsum", tag="logsum")
                            nc.scalar.activation(logsum[:], rowsum[:],
                                                 mybir.ActivationFunctionType.Ln)
                            bias2 = scv.tile([P, 1], F32, name="bias2", tag="bias2")
                            nc.vector.tensor_tensor(bias2[:], negmax[:], logsum[:],
                                                    op=mybir.AluOpType.subtract)
                            nc.scalar.activation(probs[:], scs[:],
                                                 mybir.ActivationFunctionType.Exp,
                                                 bias=bias2[:])
                            for tcn in range(NSB):
                                transpose_to(pT[tcn][:, sb4 * P:(sb4 + 1) * P],
                                             probs[:, tcn * P:(tcn + 1) * P], P, P)

                        psx = ppc.tile([P, 512], F32, name="ps_ctx", tag="ps_ctx")
                        for tcn in range(NSB):
                            nc.tensor.matmul(psx[:],
                                             _mm(veffT[:, tcn * P:(tcn + 1) * P]),
                                             _mm(pT[tcn][:]),
                                             start=(tcn == 0), stop=(tcn == NSB - 1))
                        nc.vector.tensor_copy(
                            ctxT[h][:, shalf * 512:(shalf + 1) * 512], psx[:])

            # ---------- PHASE 4: output projection (partial) ----------
            with tc.tile_pool(name="wo_pool", bufs=1) as wp4, \
                 tc.tile_pool(name="sc4", bufs=2) as sc4, \
                 tc.tile_pool(name="pp4", bufs=4, space="PSUM") as pp:
                wo = []
                for h in range(HPC):
                    t = wp4.tile([P, DIM], MMDT, name=f"wo{h}", tag=f"wo{h}")
                    nc.sync.dma_start(t[:], woutT[h * P:(h + 1) * P, :])
                    wo.append(t)

                for sb in range(NSB):
                    for nb in range(4):
                        pso = pp.tile([P, 512], F32, name="ps_out", tag="ps")
                        for h in range(HPC):
                            nc.tensor.matmul(pso[:],
                                             _mm(ctxT[h][:, sb * P:(sb + 1) * P]),
                                             _mm(wo[h][:, nb * 512:(nb + 1) * 512]),
                                             start=(h == 0), stop=False)
                        nc.tensor.matmul(pso[:], _mm(ones_row[:]),
                                         _mm(t_outb[:, nb * 512:(nb + 1) * 512]),
                                         start=False, stop=True)
                        ot = sc4.tile([P, 512], F32, name="ot", tag="ot")
                        nc.vector.tensor_copy(ot[:], pso[:])
                        nc.sync.dma_start(out_p[sb * P:(sb + 1) * P,
                                                nb * 512:(nb + 1) * 512], ot[:])

    nc.compile()
    return nc


def prep_core_inputs(x, mask, q_down_w, q_down_b, q_norm_scale, q_up_w, q_up_b,
                     kv_down_w, kv_down_b, kv_norm_scale, kv_up_w, out_w, out_b):
    """Host-side shard/transpose prep. Returns list of 8 in_maps."""
    f = np.float32
    inv = f(1.0 / np.sqrt(QK_TOTAL))

    wqdT = np.ascontiguousarray(q_down_w.T, dtype=f)
    wkvdT = np.ascontiguousarray(kv_down_w.T, dtype=f)
    qdb = q_down_b.reshape(1, -1).astype(f)
    kvdb = kv_down_b.reshape(1, -1).astype(f)

    q_up_eff = (q_up_w.astype(f) * q_norm_scale[None, :].astype(f)) * inv
    qub_eff = (q_up_b.astype(f) * inv).reshape(H, QK_TOTAL)

    wk_all = kv_up_w[:H * QK_STATIC].reshape(H, QK_STATIC, KV_RANK).astype(f)
    wv_all = kv_up_w[-H * V_DIM:].reshape(H, V_DIM, KV_RANK).astype(f)
    kvs = kv_norm_scale.astype(f)

    # rope tables (positions 0..SEQ-1)
    invf = 1.0 / (10000.0 ** (np.arange(0, QK_ROT, 2, dtype=np.float64) / QK_ROT))
    freqs = np.arange(SEQ, dtype=np.float64)[:, None] * invf[None, :]
    cosf = np.concatenate([np.cos(freqs), np.cos(freqs)], axis=-1).astype(f)
    sinf = np.concatenate([np.sin(freqs), np.sin(freqs)], axis=-1).astype(f)
    sinhr = sinf.copy()
    sinhr[:, :QK_ROT // 2] *= -1.0  # pre-negated lower half

    in_maps = []
    for c in range(N_CORES):
        b, g = c // 4, c % 4
        hs = slice(g * HPC, (g + 1) * HPC)
        wqupT = np.ascontiguousarray(
            q_up_eff.reshape(H, QK_TOTAL, Q_RANK)[hs].reshape(HPC * QK_TOTAL, Q_RANK).T,
            dtype=f)
        qub = qub_eff[hs].reshape(1, HPC * QK_TOTAL)
        wkT = np.ascontiguousarray(
            (wk_all[hs] * kvs[None, None, :]).transpose(0, 2, 1), dtype=f)
        wvT = np.ascontiguousarray(
            (wv_all[hs] * kvs[None, None, :]).transpose(0, 2, 1), dtype=f)
        woutT = np.ascontiguousarray(
            out_w[:, g * HPC * V_DIM:(g + 1) * HPC * V_DIM].T, dtype=f)
        outb = (out_b if g == 0 else np.zeros_like(out_b)).reshape(1, -1).astype(f)
        in_maps.append({
            "xT": np.ascontiguousarray(x[b].T, dtype=f),
            "mask": np.ascontiguousarray(mask[b], dtype=f),
            "wqdT": wqdT, "qdb": qdb,
            "wkvdT": wkvdT, "kvdb": kvdb,
            "wqupT": wqupT, "qub": np.ascontiguousarray(qub),
            "wkT": wkT, "wvT": wvT,
            "woutT": woutT, "outb": outb,
            "cosf": cosf, "sinhr": sinhr,
            "ones_in": np.ones((1, P), dtype=f),
        })
    return in_maps


_NC_CACHE = None


def kernel(**inputs):
    global _NC_CACHE
    x = np.asarray(inputs["x"], dtype=np.float32)
    args = {k: np.asarray(v) for k, v in inputs.items()
            if k not in ("x", "start_pos")}
    in_maps = prep_core_inputs(x=x, **{k: args[k] for k in (
        "mask", "q_down_w", "q_down_b", "q_norm_scale", "q_up_w", "q_up_b",
        "kv_down_w", "kv_down_b", "kv_norm_scale", "kv_up_w", "out_w", "out_b")})
    if _NC_CACHE is None:
        _NC_CACHE = build_kernel()
    res = run_bass_kernel_spmd(_NC_CACHE, in_maps, list(range(N_CORES))).results
    out = np.zeros((BS, SEQ, DIM), dtype=np.float32)
    for c in range(N_CORES):
        out[c // 4] += res[c]["out_p"]
    return out

